# revision 3
# baseline (speedup 1.0000x reference)
"""CovariantEvolutionBlock Trainium2 kernel.

Strategy: token-parallel over B*L across 8 cores (512 tokens/core), zero
collectives. Each core recomputes full-batch K/V for attention (inputs are
rotated per-core so "own" tokens are always columns 0:512; sigmoid attention
is permutation-invariant over keys). Activations are kept feature-major
[dims, tokens] on-chip so matmul chains need no transposes; weights are
pre-transposed/cast to bf16 on the host. All matmuls are bf16 with fp32 PSUM
accumulation.
"""

import sys

try:
    import concourse.bass as bass  # noqa: F401
except ImportError:
    sys.path.insert(0, "/opt/trn_rl_repo")

import numpy as np
import ml_dtypes

import concourse.bacc as bacc
import concourse.tile as tile
import concourse.mybir as mybir
from concourse.bass_utils import run_bass_kernel_spmd

F32 = mybir.dt.float32
BF16 = mybir.dt.bfloat16
AF = mybir.ActivationFunctionType

B, L, D, H, HD = 2, 2048, 1024, 16, 64
EPS = 1e-6
NCORES = 8
TOK = 512          # own tokens per core
KEYS = 2048        # keys per batch
KC = D // 128      # 8 feature chunks of 128
NTB = KEYS // TOK  # 4 token blocks per batch
OBW = 256          # out-block width (2 m-chunks) per psum tile


def _bias_ap(dram_ap):
    # [dim] -> [128, dim//128]: tile[p, c] = bias[c*128 + p]
    return dram_ap.rearrange("(c p) -> p c", p=128)


def build_program(dt_val: float, temp_val: float):
    nc = bacc.Bacc("TRN2", target_bir_lowering=False, debug=False,
                   num_devices=NCORES)

    d_in = {}
    for name, shape, dt in [
        ("zT", [D, KEYS], F32), ("cT", [D, KEYS], F32),
        ("fw1T", [D, 2 * D], BF16), ("fw2T", [2 * D, D], BF16),
        ("gw1T", [2 * D, D], BF16), ("gw2T", [D, D], BF16),
        ("qwT", [2 * D, D], BF16), ("kwT", [2 * D, D], BF16),
        ("vwT", [D, D], BF16), ("owT", [D, D], BF16),
        ("cuw1T", [3 * D, 2 * D], BF16), ("cuw2T", [2 * D, D], BF16),
        ("mw1T", [D, 4 * D], BF16), ("mw2T", [4 * D, D], BF16),
        ("fb1", [2 * D], F32), ("fb2", [D], F32),
        ("gb1", [D], F32), ("gb2", [D], F32),
        ("cub1", [2 * D], F32), ("cub2", [D], F32),
        ("mb1", [4 * D], F32), ("mb2", [D], F32),
        ("wz", [D], F32), ("wc", [D], F32), ("wmlp", [D], F32),
    ]:
        d_in[name] = nc.dram_tensor(name, shape, dt, kind="ExternalInput").ap()

    z2T_d = nc.dram_tensor("z2T", [D, TOK], F32, kind="ExternalOutput").ap()
    connT_d = nc.dram_tensor("connT", [D, TOK], F32, kind="ExternalOutput").ap()

    sig_scale = float(temp_val) * (HD ** -0.5)

    with tile.TileContext(nc) as tc:
        _emit(nc, tc, d_in, z2T_d, connT_d, float(dt_val), sig_scale)
    nc.compile()
    return nc


def _emit(nc, tc, d_in, z2T_d, connT_d, dt_val, sig_scale):
    from contextlib import ExitStack

    ctx = ExitStack()
    with ctx:
        # ---------- persistent pools ----------
        const = ctx.enter_context(tc.tile_pool(name="const", bufs=1))
        persist = ctx.enter_context(tc.tile_pool(name="persist", bufs=1))
        wpool = ctx.enter_context(tc.tile_pool(name="wpool", bufs=4))
        ps_lin = ctx.enter_context(
            tc.tile_pool(name="ps_lin", bufs=2, space="PSUM"))

        # constants: biases, rms weights, ones
        bias = {}
        for name in ["fb1", "fb2", "gb1", "gb2", "cub1", "cub2", "mb1", "mb2"]:
            n = d_in[name].shape[0]
            t = const.tile([128, n // 128], F32, tag=name)
            nc.sync.dma_start(out=t[:], in_=_bias_ap(d_in[name]))
            bias[name] = t
        wcol = {}
        for name in ["wz", "wc", "wmlp"]:
            t = const.tile([128, KC], F32, tag=name)
            nc.sync.dma_start(out=t[:], in_=_bias_ap(d_in[name]))
            wcol[name] = t
        ones_col = const.tile([128, 1], BF16, tag="ones")
        nc.vector.memset(ones_col[:], 1.0)
        eps1 = const.tile([1, 1], F32, tag="eps1")
        nc.vector.memset(eps1[:], EPS)

        # persistent activations (own tokens, feature-major, bf16)
        cn_own = persist.tile([128, KC, TOK], BF16, tag="cn_own")
        attnT = persist.tile([128, KC, TOK], BF16, tag="attnT")
        mlp = ctx.enter_context(tc.tile_pool(name="mlp", bufs=1))
        # (late-phase tiles go in mlp2, opened in phase 4)

        # ---------- generic feature-major linear ----------
        def linear_fm(wT_d, n_in, n_out, rhs_fn, evict_fn, wtag="w"):
            # out[m-chunk] = sum_k wT[k,m].T @ rhs(k); evict_fn(mc, psum_ap)
            nob = n_out // OBW
            kcn = n_in // 128
            for ob in range(nob):
                ps = ps_lin.tile([128, 2, 512], F32, tag="lin")
                for k in range(kcn):
                    w = wpool.tile([128, OBW], BF16, tag=wtag, bufs=8)
                    nc.sync.dma_start(
                        out=w[:],
                        in_=wT_d[k * 128:(k + 1) * 128, ob * OBW:(ob + 1) * OBW])
                    for m in range(2):
                        nc.tensor.matmul(
                            ps[:, m, :TOK], w[:, m * 128:(m + 1) * 128],
                            rhs_fn(k), start=(k == 0), stop=(k == kcn - 1))
                for m in range(2):
                    evict_fn(ob * 2 + m, ps[:, m, :TOK])

        # ---------- phase 1+2: norms, K, V, Q ----------
        with tc.tile_pool(name="kvq", bufs=1) as kvq:
            KT = kvq.tile([128, KC, KEYS], BF16, tag="KT")
            V_sb = kvq.tile([128, H, H, HD + 1], BF16, tag="V")
            QT_z = kvq.tile([128, H, TOK], BF16, tag="QT")
            zn_own = kvq.tile([128, KC, TOK], BF16, tag="zn_own")
            nc.vector.memset(QT_z[:], 0.0)
            nc.vector.memset(V_sb[:, :, :, HD:HD + 1], 1.0)
            norm_scope = ExitStack()
            nrm = norm_scope.enter_context(tc.tile_pool(name="nrm", bufs=1))
            xrawp = norm_scope.enter_context(
                tc.tile_pool(name="xraw", bufs=3))
            ps_ss = norm_scope.enter_context(
                tc.tile_pool(name="ps_ss", bufs=2, space="PSUM"))

            def norm_block(xT_d, w_t, dst, raw_dst=None):
                # one token-block norm: returns nothing; writes normed bf16
                # chunks into dst [128, KC, TOK]
                ss = ps_ss.tile([1, TOK], F32, tag="ss")
                xb = nrm.tile([128, KC, TOK], BF16, tag="xbf", bufs=2)
                for k in range(KC):
                    xf = xrawp.tile([128, TOK], F32, tag="xf", bufs=2)
                    nc.sync.dma_start(out=xf[:], in_=xT_d[k])
                    nc.vector.tensor_copy(xb[:, k, :], xf[:])
                    sq = xrawp.tile([128, TOK], BF16, tag="sq", bufs=2)
                    nc.vector.tensor_mul(sq[:], xb[:, k, :], xb[:, k, :])
                    nc.tensor.matmul(ss[:], ones_col[:], sq[:],
                                     start=(k == 0), stop=(k == KC - 1))
                sf = xrawp.tile([1, TOK], F32, tag="sf", bufs=1)
                nc.scalar.activation(sf[:], ss[:], AF.Sqrt,
                                     bias=eps1[:], scale=1.0 / D)
                nc.vector.reciprocal(sf[:], sf[:])
                bc = xrawp.tile([128, TOK], F32, tag="bc", bufs=2)
                nc.gpsimd.partition_broadcast(bc[:], sf[0:1, :])
                for k in range(KC):
                    nc.vector.scalar_tensor_tensor(
                        dst[:, k, :], bc[:], w_t[:, k:k + 1], xb[:, k, :],
                        op0=mybir.AluOpType.mult, op1=mybir.AluOpType.mult)

            for tb in range(NTB):
                cols = slice(tb * TOK, (tb + 1) * TOK)
                zslices = [d_in["zT"][k * 128:(k + 1) * 128, cols]
                           for k in range(KC)]
                cslices = [d_in["cT"][k * 128:(k + 1) * 128, cols]
                           for k in range(KC)]
                if tb == 0:
                    zn_tb, cn_tb = zn_own, cn_own
                else:
                    zn_tb = nrm.tile([128, KC, TOK], BF16, tag="zn_tb",
                                     bufs=2)
                    cn_tb = nrm.tile([128, KC, TOK], BF16, tag="cn_tb",
                                     bufs=2)
                norm_block(zslices, wcol["wz"], zn_tb)

                # V first: needs only zn, overlaps the c-norm
                for kc4 in range(4):
                    kcg = tb * 4 + kc4
                    ps = ps_lin.tile([128, 2, 512], F32, tag="lin")
                    for k in range(KC):
                        lhs = zn_tb[:, k, kc4 * 128:(kc4 + 1) * 128]
                        for vb in range(2):
                            vw = wpool.tile([128, 512], BF16, tag="vw",
                                            bufs=3)
                            nc.sync.dma_start(
                                out=vw[:],
                                in_=d_in["vwT"][k * 128:(k + 1) * 128,
                                                vb * 512:(vb + 1) * 512])
                            nc.tensor.matmul(
                                ps[:, vb, :], lhs, vw[:],
                                start=(k == 0), stop=(k == KC - 1))
                    for vb in range(2):
                        src = ps[:, vb, :].rearrange("p (h d) -> p h d", h=8)
                        nc.scalar.activation(
                            V_sb[:, kcg, vb * 8:(vb + 1) * 8, 0:HD], src,
                            AF.Copy)

                norm_block(cslices, wcol["wc"], cn_tb)

                # K for this token block -> KT[:, :, tb]
                def k_rhs(k):
                    return (zn_tb[:, k, :] if k < KC
                            else cn_tb[:, k - KC, :])

                def k_evict(mc, ps):
                    nc.scalar.activation(KT[:, mc, cols], ps, AF.Copy)

                linear_fm(d_in["kwT"], 2 * D, D, k_rhs, k_evict, wtag="kw")

                if tb == 0:
                    # Q projection (own tokens), zero-padded per head
                    def q_rhs(k):
                        return (zn_own[:, k, :] if k < KC
                                else cn_own[:, k - KC, :])

                    def q_evict(mc, ps):
                        nc.scalar.activation(
                            QT_z[0:64, 2 * mc, :], ps[0:64, :], AF.Copy)
                        nc.scalar.activation(
                            QT_z[64:128, 2 * mc + 1, :], ps[64:128, :],
                            AF.Copy)

                    linear_fm(d_in["qwT"], 2 * D, D, q_rhs, q_evict,
                              wtag="qw")

            norm_scope.close()

            # ---------- f MLP early: interleaves with attention ----------
            fh = mlp.tile([128, 2 * KC, TOK], BF16, tag="fh")
            dzl_b = mlp.tile([128, KC, TOK], BF16, tag="dzl")

            def evict_silu(dst, ps, bias_ap):
                # silu(x) = x * sigmoid(x); CoreSim has no native Silu
                sg = mlp.tile([128, TOK], BF16, tag="sg", bufs=3)
                nc.scalar.activation(sg[:], ps, AF.Sigmoid, bias=bias_ap)
                nc.vector.scalar_tensor_tensor(
                    dst, ps, bias_ap, sg[:],
                    op0=mybir.AluOpType.add, op1=mybir.AluOpType.mult)

            def f1_evict(mc, ps):
                evict_silu(fh[:, mc, :], ps, bias["fb1"][:, mc:mc + 1])

            linear_fm(d_in["fw1T"], D, 2 * D,
                      lambda k: zn_own[:, k, :], f1_evict)

            def f2_evict(mc, ps):
                nc.vector.tensor_scalar_add(dzl_b[:, mc, :], ps,
                                            bias["fb2"][:, mc:mc + 1])

            linear_fm(d_in["fw2T"], 2 * D, D,
                      lambda k: fh[:, k, :], f2_evict)

            # ---------- phase 3: sigmoid attention ----------
            with (
                tc.tile_pool(name="rel", bufs=1) as relp,
                tc.tile_pool(name="att_s", bufs=2) as attsp,
                tc.tile_pool(name="ps_sc", bufs=2, space="PSUM") as ps_sc,
                tc.tile_pool(name="ps_av", bufs=2, space="PSUM") as ps_av,
            ):
                for h in range(H):
                    rel = relp.tile([128, H, TOK], BF16, tag="rel")
                    for kc in range(H):
                        sc = ps_sc.tile([128, TOK], F32, tag="sc")
                        nc.tensor.matmul(
                            sc[:], KT[:, h // 2, kc * 128:(kc + 1) * 128],
                            QT_z[:, h, :], start=True, stop=True)
                        nc.scalar.activation(rel[:, kc, :], sc[:], AF.Sigmoid,
                                             scale=sig_scale)
                    av = ps_av.tile([65, TOK], F32, tag="av")
                    for kc in range(H):
                        nc.tensor.matmul(av[:], V_sb[:, kc, h, :],
                                         rel[:, kc, :],
                                         start=(kc == 0), stop=(kc == H - 1))
                    # rel_sum = clip(row 64, 1, inf); recip; bcast; divide
                    rs = attsp.tile([1, TOK], F32, tag="rs")
                    nc.vector.tensor_scalar_max(rs[0:1, :], av[64:65, :], 1.0)
                    nc.vector.reciprocal(rs[0:1, :], rs[0:1, :])
                    bcv = attsp.tile([64, TOK], F32, tag="bcv")
                    nc.gpsimd.partition_broadcast(bcv[:], rs[0:1, :])
                    po = (h % 2) * 64
                    nc.vector.tensor_mul(attnT[po:po + 64, h // 2, :],
                                         av[0:64, :], bcv[:])

        # ---------- phase 4: dz MLPs, o-proj, cu, final MLP ----------
        with (
            tc.tile_pool(name="mlp2", bufs=1) as mlp2,
            tc.tile_pool(name="outp", bufs=2) as outp,
            tc.tile_pool(name="ps_ss2", bufs=2, space="PSUM") as ps_ss2,
        ):
            # hid: du(16) -> mh(32) share one 32KB slot via tag
            gh = mlp2.tile([128, KC, TOK], BF16, tag="mid8")
            s_b = mlp2.tile([128, KC, TOK], BF16, tag="s_b")
            s_f = mlp2.tile([128, KC, TOK], F32, tag="s_f")

            # stage raw connection (bf16) early for cu1
            c_raw = mlp2.tile([128, KC, TOK], BF16, tag="c_raw")
            for k in range(KC):
                ct = mlp2.tile([128, TOK], F32, tag="zot", bufs=2)
                nc.sync.dma_start(
                    out=ct[:], in_=d_in["cT"][k * 128:(k + 1) * 128, 0:TOK])
                nc.vector.tensor_copy(c_raw[:, k, :], ct[:])

            # gh = tanh(cat(cn, dzl) @ g_w1.T + gb1)
            def g1_evict(mc, ps):
                nc.scalar.activation(gh[:, mc, :], ps, AF.Tanh,
                                     bias=bias["gb1"][:, mc:mc + 1])

            linear_fm(d_in["gw1T"], 2 * D, D,
                      lambda k: cn_own[:, k, :] if k < KC
                      else dzl_b[:, k - KC, :], g1_evict)

            # s = dzl + (gh @ g_w2.T + gb2)   (dz = dt*s)
            def g2_evict(mc, ps):
                nc.vector.scalar_tensor_tensor(
                    s_f[:, mc, :], ps, bias["gb2"][:, mc:mc + 1],
                    dzl_b[:, mc, :], op0=mybir.AluOpType.add,
                    op1=mybir.AluOpType.add)
                nc.vector.tensor_copy(s_b[:, mc, :], s_f[:, mc, :])

            linear_fm(d_in["gw2T"], D, D, lambda k: gh[:, k, :], g2_evict)

            # ctx = attn @ o_w.T ; z1 = z + dt*s + ctx
            z1_f = mlp2.tile([128, KC, TOK], F32, tag="z1f")
            z1_b = mlp2.tile([128, KC, TOK], BF16, tag="z1b")

            def o_evict(mc, ps):
                zot = mlp2.tile([128, TOK], F32, tag="zot", bufs=2)
                nc.sync.dma_start(
                    out=zot[:],
                    in_=d_in["zT"][mc * 128:(mc + 1) * 128, 0:TOK])
                t = mlp2.tile([128, TOK], F32, tag="t_z1", bufs=2)
                nc.vector.scalar_tensor_tensor(
                    t[:], s_f[:, mc, :], dt_val, ps,
                    op0=mybir.AluOpType.mult, op1=mybir.AluOpType.add)
                nc.vector.tensor_add(z1_f[:, mc, :], t[:], zot[:])
                nc.vector.tensor_copy(z1_b[:, mc, :], z1_f[:, mc, :])

            linear_fm(d_in["owT"], D, D,
                      lambda k: attnT[:, k, :], o_evict)

            # cu: du = silu(cat(c, z1, dt*s) @ cu_w1.T + cub1)
            du = mlp2.tile([128, 32, TOK], BF16, tag="hid")

            def cu1_rhs(k):
                if k < KC:
                    return c_raw[:, k, :]
                if k < 2 * KC:
                    return z1_b[:, k - KC, :]
                return s_b[:, k - 2 * KC, :]

            def cu1_evict(mc, ps):
                evict_silu(du[:, mc, :], ps, bias["cub1"][:, mc:mc + 1])

            linear_fm(d_in["cuw1T"], 3 * D, 2 * D, cu1_rhs, cu1_evict)

            # conn_new = c + (du @ cu_w2.T + cub2)
            def cu2_evict(mc, ps):
                ct = mlp2.tile([128, TOK], F32, tag="zot", bufs=2)
                nc.sync.dma_start(
                    out=ct[:], in_=d_in["cT"][mc * 128:(mc + 1) * 128, 0:TOK])
                co = outp.tile([128, TOK], F32, tag="co")
                nc.vector.scalar_tensor_tensor(
                    co[:], ps, bias["cub2"][:, mc:mc + 1], ct[:],
                    op0=mybir.AluOpType.add, op1=mybir.AluOpType.add)
                nc.sync.dma_start(
                    out=connT_d[mc * 128:(mc + 1) * 128, :], in_=co[:])

            linear_fm(d_in["cuw2T"], 2 * D, D,
                      lambda k: du[:, k, :], cu2_evict)

            # z1n = rms(z1) * wmlp
            z1n = mlp2.tile([128, KC, TOK], BF16, tag="mid8")
            ss = ps_ss2.tile([1, TOK], F32, tag="ss2")
            for k in range(KC):
                sq = mlp2.tile([128, TOK], BF16, tag="sq2", bufs=2)
                nc.vector.tensor_mul(sq[:], z1_b[:, k, :], z1_b[:, k, :])
                nc.tensor.matmul(ss[:], ones_col[:], sq[:],
                                 start=(k == 0), stop=(k == KC - 1))
            sf = mlp2.tile([1, TOK], F32, tag="sf2")
            nc.scalar.activation(sf[:], ss[:], AF.Sqrt, bias=eps1[:],
                                 scale=1.0 / D)
            nc.vector.reciprocal(sf[:], sf[:])
            bc2 = mlp2.tile([128, TOK], F32, tag="bc2")
            nc.gpsimd.partition_broadcast(bc2[:], sf[0:1, :])
            for k in range(KC):
                nc.vector.scalar_tensor_tensor(
                    z1n[:, k, :], bc2[:], wcol["wmlp"][:, k:k + 1],
                    z1_b[:, k, :], op0=mybir.AluOpType.mult,
                    op1=mybir.AluOpType.mult)

            # mh = silu(z1n @ m_w1.T + mb1)
            mh = mlp2.tile([128, 32, TOK], BF16, tag="hid")

            def m1_evict(mc, ps):
                evict_silu(mh[:, mc, :], ps, bias["mb1"][:, mc:mc + 1])

            linear_fm(d_in["mw1T"], D, 4 * D,
                      lambda k: z1n[:, k, :], m1_evict)

            # z2 = z1 + (mh @ m_w2.T + mb2)
            def m2_evict(mc, ps):
                zo = outp.tile([128, TOK], F32, tag="zo")
                nc.vector.scalar_tensor_tensor(
                    zo[:], ps, bias["mb2"][:, mc:mc + 1], z1_f[:, mc, :],
                    op0=mybir.AluOpType.add, op1=mybir.AluOpType.add)
                nc.sync.dma_start(
                    out=z2T_d[mc * 128:(mc + 1) * 128, :], in_=zo[:])

            linear_fm(d_in["mw2T"], 4 * D, D,
                      lambda k: mh[:, k, :], m2_evict)


_CACHE = {}
_RUN_KWARGS = {}
_LAST_RESULT = None


def _prep_shared(inputs):
    bf = ml_dtypes.bfloat16

    def t(x, dt=bf):
        return np.ascontiguousarray(np.asarray(x, np.float32).T).astype(dt)

    dt_val = float(np.asarray(inputs["dt"]))
    cu1 = np.asarray(inputs["cu_w1"], np.float32).copy()
    cu1[:, 2 * D:] *= dt_val  # fold dz = dt*s into cu_w1's dz block
    shared = {
        "fw1T": t(inputs["f_w1"]), "fw2T": t(inputs["f_w2"]),
        "gw1T": t(inputs["g_w1"]), "gw2T": t(inputs["g_w2"]),
        "qwT": t(inputs["q_w"]), "kwT": t(inputs["k_w"]),
        "vwT": t(inputs["v_w"]),
        "owT": t(inputs["o_w"]),
        "cuw1T": np.ascontiguousarray(cu1.T).astype(bf),
        "cuw2T": t(inputs["cu_w2"]),
        "mw1T": t(inputs["m_w1"]), "mw2T": t(inputs["m_w2"]),
    }
    for name, key in [("fb1", "f_b1"), ("fb2", "f_b2"), ("gb1", "g_b1"),
                      ("gb2", "g_b2"), ("cub1", "cu_b1"), ("cub2", "cu_b2"),
                      ("mb1", "m_b1"), ("mb2", "m_b2"), ("wz", "w_z"),
                      ("wc", "w_c"), ("wmlp", "w_mlp")]:
        shared[name] = np.ascontiguousarray(np.asarray(inputs[key], np.float32))
    return shared


def kernel(**inputs):
    z = np.asarray(inputs["z"], np.float32)
    conn = np.asarray(inputs["connection"], np.float32)
    dt_val = float(np.asarray(inputs["dt"]))
    temp_val = float(np.asarray(inputs["temp"]))

    key = (dt_val, temp_val)
    if key not in _CACHE:
        _CACHE[key] = build_program(dt_val, temp_val)
    nc = _CACHE[key]

    shared = _prep_shared(inputs)
    zT = [np.ascontiguousarray(z[b].T) for b in range(B)]
    cT = [np.ascontiguousarray(conn[b].T) for b in range(B)]

    in_maps = []
    for c in range(NCORES):
        b, tb = divmod(c, NTB)
        m = dict(shared)
        m["zT"] = np.ascontiguousarray(np.roll(zT[b], -tb * TOK, axis=1))
        m["cT"] = np.ascontiguousarray(np.roll(cT[b], -tb * TOK, axis=1))
        in_maps.append(m)

    res = run_bass_kernel_spmd(nc, in_maps, list(range(NCORES)), **_RUN_KWARGS)
    global _LAST_RESULT
    _LAST_RESULT = res

    z2 = np.empty((B, L, D), np.float32)
    conn_new = np.empty((B, L, D), np.float32)
    for c in range(NCORES):
        b, tb = divmod(c, NTB)
        sl = slice(tb * TOK, (tb + 1) * TOK)
        z2[b, sl, :] = res.results[c]["z2T"].T
        conn_new[b, sl, :] = res.results[c]["connT"].T
    return z2, conn_new, z



# revision 30
# speedup vs baseline: 1.5929x; 1.5929x over previous
"""CovariantEvolutionBlock Trainium2 kernel (v2).

Strategy: token-parallel over B*L across 8 cores (512 own tokens/core), zero
collectives; inputs rotated per-core so own tokens are keys block 0. All
activations kept feature-major [128, chunk, tokens] on-chip.

v2 changes vs v1 (1.33ms baseline):
- Weights are host-pre-tiled to [128, K/128, M] and DMA'd exactly once in a
  few large contiguous transfers (v1 streamed 64KB tiles, re-reading K/V
  weights 4x: 133MB -> ~62MB HBM traffic).
- fp8 (e4m3) + DoubleRow matmuls for Q/K/V/scores/attnV/f/g paths (halves
  matmul instruction count there; verified 3.5e-3 end-to-end rel err in
  numpy vs the 2e-2 budget). o/cu/m paths stay bf16 (fp8 there breaks the
  error budget since delta/mlp feed residuals undamped).
- Attention scores packed two heads per PE pass via tile_position row
  packing (contraction is only 64 wide per head).
- Norm/rel_sum chains avoid 1-partition vector ops and gpsimd broadcasts:
  scalar copy/sqrt -> K=1 fp32 matmul broadcast -> vector reciprocal on
  [64|128, 512] tiles.
- f1 MLP interleaved under attention so the 256 sigmoid evictions (scalar
  engine) overlap tensor work.
"""

import sys

try:
    import concourse.bass as bass  # noqa: F401
except ImportError:
    sys.path.insert(0, "/opt/trn_rl_repo")

import numpy as np
import ml_dtypes

import concourse.bacc as bacc
import concourse.tile as tile
import concourse.mybir as mybir
from concourse.bass_utils import run_bass_kernel_spmd

F32 = mybir.dt.float32
BF16 = mybir.dt.bfloat16
FP8 = mybir.dt.float8e4
AF = mybir.ActivationFunctionType
DR = mybir.MatmulPerfMode.DoubleRow

B, L, D, H, HD = 2, 2048, 1024, 16, 64
EPS = 1e-6
NCORES = 8
TOK = 512          # own tokens per core
KEYS = 2048        # keys per batch
KC = D // 128      # 8 feature chunks of 128
NTB = KEYS // TOK  # 4 token blocks per batch


def _bias_ap(dram_ap):
    # [dim] -> [128, dim//128]: tile[p, c] = bias[c*128 + p]
    return dram_ap.rearrange("(c p) -> p c", p=128)


def build_program(dt_val: float, temp_val: float):
    nc = bacc.Bacc("TRN2", target_bir_lowering=False, debug=False,
                   num_devices=NCORES)

    d = {}
    for name, shape, dtp in [
        ("zb", [128, KC, KEYS], BF16), ("cb", [128, KC, KEYS], BF16),
        ("zf", [128, KC, TOK], F32), ("cf", [128, KC, TOK], F32),
        ("qw8", [128, 16, D], FP8), ("kw8", [128, 16, D], FP8),
        ("vw8", [128, 8, D], FP8),
        ("fw18", [128, 8, 2 * D], FP8), ("fw28", [128, 16, D], FP8),
        ("gw18", [128, 16, D], FP8), ("gw28", [128, 8, D], FP8),
        ("owb", [128, 8, D], BF16),
        ("cuw1b", [128, 24, 2 * D], BF16), ("cuw2b", [128, 16, D], BF16),
        ("mw1b", [128, 8, 4 * D], BF16), ("mw2b", [128, 32, D], BF16),
        ("fb1", [2 * D], F32), ("fb2", [D], F32),
        ("gb1", [D], F32), ("gb2", [D], F32),
        ("cub1", [2 * D], F32), ("cub2", [D], F32),
        ("mb1", [4 * D], F32), ("mb2", [D], F32),
        ("wz", [D], F32), ("wc", [D], F32), ("wmlp", [D], F32),
    ]:
        d[name] = nc.dram_tensor(name, shape, dtp, kind="ExternalInput").ap()

    z2o_d = nc.dram_tensor("z2o", [128, KC, TOK], F32, kind="ExternalOutput").ap()
    cno_d = nc.dram_tensor("cno", [128, KC, TOK], F32, kind="ExternalOutput").ap()

    sig_scale = float(temp_val) * (HD ** -0.5)

    with tile.TileContext(nc) as tc:
        _emit(nc, tc, d, z2o_d, cno_d, float(dt_val), sig_scale)
    nc.compile()
    return nc


def _emit(nc, tc, d, z2o_d, cno_d, dt_val, sig_scale):
    from contextlib import ExitStack

    ctx = ExitStack()
    with ctx:
        # ---------- pools alive the whole program ----------
        const = ctx.enter_context(tc.tile_pool(name="const", bufs=1))
        persist = ctx.enter_context(tc.tile_pool(name="persist", bufs=1))

        bias = {}
        for name in ["fb1", "fb2", "gb1", "gb2", "cub1", "cub2", "mb1", "mb2"]:
            n = d[name].shape[0]
            t = const.tile([128, n // 128], F32, tag=name)
            nc.sync.dma_start(out=t[:], in_=_bias_ap(d[name]))
            bias[name] = t
        wcol = {}
        for name in ["wz", "wc", "wmlp"]:
            t = const.tile([128, KC], F32, tag=name)
            nc.sync.dma_start(out=t[:], in_=_bias_ap(d[name]))
            wcol[name] = t
        ones_col = const.tile([128, 1], BF16, tag="ones_col")
        nc.vector.memset(ones_col[:], 1.0)
        ones_bc = const.tile([1, 128], F32, tag="ones_bc")
        nc.vector.memset(ones_bc[:], 1.0)
        eps1 = const.tile([1, 1], F32, tag="eps1")
        nc.vector.memset(eps1[:], EPS)

        # persistent across phase boundaries (outlive the attn scope)
        z1b = persist.tile([128, KC, TOK], BF16, tag="z1b")
        z1f = persist.tile([128, KC, TOK], F32, tag="z1f")
        # s_b16 spans phases D..E only; its pool closes before the final MLP
        de_scope = ctx.enter_context(ExitStack())
        de_pool = de_scope.enter_context(tc.tile_pool(name="de", bufs=1))
        s_b16 = de_pool.tile([128, KC, TOK], BF16, tag="s_b16")

        # ---------- helpers ----------
        def lin8(wt, n_in, n_out, rhs_fn, evict_fn, pool, psbufs=2):
            # fp8 DoubleRow linear: out[mc] = sum_i wt[:,2i:2i+2,mc].T2 @ rhs(i)
            ndr = n_in // 256
            for ob in range(n_out // 256):
                ps = pool.tile([128, 2, TOK], F32, tag="lin", bufs=psbufs)
                for m in range(2):
                    mc = ob * 2 + m
                    for i in range(ndr):
                        nc.tensor.matmul(
                            ps[:, m, :],
                            wt[:, 2 * i:2 * i + 2, mc * 128:(mc + 1) * 128],
                            rhs_fn(i), start=(i == 0), stop=(i == ndr - 1),
                            perf_mode=DR)
                for m in range(2):
                    evict_fn(ob * 2 + m, ps[:, m, :])

        def lin16(wt, n_in, n_out, rhs_fn, evict_fn, pool, psbufs=2):
            # bf16 linear, weights resident in SBUF
            kcn = n_in // 128
            for ob in range(n_out // 256):
                ps = pool.tile([128, 2, TOK], F32, tag="lin", bufs=psbufs)
                for m in range(2):
                    mc = ob * 2 + m
                    for k in range(kcn):
                        nc.tensor.matmul(
                            ps[:, m, :], wt[:, k, mc * 128:(mc + 1) * 128],
                            rhs_fn(k), start=(k == 0), stop=(k == kcn - 1))
                for m in range(2):
                    evict_fn(ob * 2 + m, ps[:, m, :])

        def evict_silu(dst, ps, bias_ap, sg_pool):
            sg = sg_pool.tile([128, TOK], BF16, tag="sg", bufs=2)
            nc.scalar.activation(sg[:], ps, AF.Sigmoid, bias=bias_ap)
            nc.vector.scalar_tensor_tensor(
                dst, ps, bias_ap, sg[:],
                op0=mybir.AluOpType.add, op1=mybir.AluOpType.mult)

        # ---------- phase A: norms, Q, K, V ----------
        with tc.tile_pool(name="attn_sc", bufs=1) as attn_sc:
            KT8 = attn_sc.tile([128, KC, KEYS], FP8, tag="KT8")
            V8 = attn_sc.tile([128, 16, H, 66], FP8, tag="V8")
            QT8 = attn_sc.tile([128, KC, TOK], FP8, tag="QT8")
            zn_own = attn_sc.tile([128, KC, TOK], FP8, tag="zn_own")
            cn_own = attn_sc.tile([128, KC, TOK], FP8, tag="cn_own")
            attnT = attn_sc.tile([128, KC, TOK], BF16, tag="attnT")
            nc.vector.memset(V8[:, :, :, 64:65], 1.0)

            norm_scope = ExitStack()
            nrm = norm_scope.enter_context(tc.tile_pool(name="nrm", bufs=1))
            tmpA = norm_scope.enter_context(tc.tile_pool(name="tmpA", bufs=1))
            ps_ss = norm_scope.enter_context(
                tc.tile_pool(name="ps_ss", bufs=2, space="PSUM"))
            ps_bc = norm_scope.enter_context(
                tc.tile_pool(name="ps_bc", bufs=2, space="PSUM"))
            ps_A = norm_scope.enter_context(
                tc.tile_pool(name="ps_A", bufs=1, space="PSUM"))

            def norm_block(xraw, w_t, dst, dst_dt_hint=None):
                # dst[:, k, :] = xraw[k] * rsqrt(mean(x^2) + eps) * w
                ss = ps_ss.tile([1, TOK], F32, tag="ss")
                for k in range(KC):
                    sq = nrm.tile([128, TOK], BF16, tag="sq", bufs=2)
                    nc.vector.tensor_mul(sq[:], xraw[:, k, :], xraw[:, k, :])
                    nc.tensor.matmul(ss[:], ones_col[:], sq[:],
                                     start=(k == 0), stop=(k == KC - 1))
                sf = nrm.tile([1, TOK], F32, tag="sf", bufs=2)
                nc.scalar.activation(sf[:], ss[:], AF.Sqrt,
                                     bias=eps1[:], scale=1.0 / D)
                bc = ps_bc.tile([128, TOK], F32, tag="bc")
                nc.tensor.matmul(bc[:], ones_bc[:], sf[:], start=True,
                                 stop=True)
                bcr = nrm.tile([128, TOK], F32, tag="bcr", bufs=2)
                nc.vector.reciprocal(bcr[:], bc[:])
                for k in range(KC):
                    nc.vector.scalar_tensor_tensor(
                        dst[:, k, :], bcr[:], w_t[:, k:k + 1], xraw[:, k, :],
                        op0=mybir.AluOpType.mult, op1=mybir.AluOpType.mult)

            def rhs_cat(zt, ct):
                def f(i):
                    if i < 4:
                        return zt[:, 2 * i:2 * i + 2, :]
                    return ct[:, 2 * (i - 4):2 * (i - 4) + 2, :]
                return f

            # --- own block (tb=0): norms + Q ---
            zraw0 = tmpA.tile([128, KC, TOK], BF16, tag="zraw", bufs=2)
            nc.sync.dma_start(out=zraw0[:], in_=d["zb"][:, :, 0:TOK])
            craw0 = tmpA.tile([128, KC, TOK], BF16, tag="craw", bufs=2)
            nc.sync.dma_start(out=craw0[:], in_=d["cb"][:, :, 0:TOK])
            norm_block(zraw0, wcol["wz"], zn_own)
            norm_block(craw0, wcol["wc"], cn_own)

            with tc.tile_pool(name="wq", bufs=1) as wq:
                qw = wq.tile([128, 16, D], FP8, tag="qw")
                nc.sync.dma_start(out=qw[:], in_=d["qw8"][:])

                def q_evict(mc, ps):
                    nc.scalar.activation(QT8[:, mc, :], ps, AF.Copy)

                lin8(qw, 2 * D, D, rhs_cat(zn_own, cn_own), q_evict, ps_A)

            # --- K/V for all 4 blocks ---
            with tc.tile_pool(name="wkv", bufs=1) as wkv:
                kw = wkv.tile([128, 16, D], FP8, tag="kw")
                nc.sync.dma_start(out=kw[:], in_=d["kw8"][:])
                vw = wkv.tile([128, 8, D], FP8, tag="vw")
                nc.sync.dma_start(out=vw[:], in_=d["vw8"][:])

                def kv_block(tb, zt, ct):
                    cols = slice(tb * TOK, (tb + 1) * TOK)
                    # V: stationary = zn tiles, moving = vw -> keys-major out
                    for kt in range(4):
                        kcg = tb * 4 + kt
                        ps = ps_A.tile([128, 2, TOK], F32, tag="lin", bufs=2)
                        for vb in range(2):
                            for i in range(4):
                                nc.tensor.matmul(
                                    ps[:, vb, :],
                                    zt[:, 2 * i:2 * i + 2,
                                       kt * 128:(kt + 1) * 128],
                                    vw[:, 2 * i:2 * i + 2,
                                       vb * TOK:(vb + 1) * TOK],
                                    start=(i == 0), stop=(i == 3),
                                    perf_mode=DR)
                        for vb in range(2):
                            src = ps[:, vb, :].rearrange(
                                "p (h d) -> p h d", h=8)
                            nc.scalar.activation(
                                V8[:, kcg, vb * 8:(vb + 1) * 8, 0:HD], src,
                                AF.Copy)

                    def k_evict(mc, ps):
                        nc.scalar.activation(KT8[:, mc, cols], ps, AF.Copy)

                    lin8(kw, 2 * D, D, rhs_cat(zt, ct), k_evict, ps_A)

                kv_block(0, zn_own, cn_own)
                for tb in range(1, NTB):
                    cols = slice(tb * TOK, (tb + 1) * TOK)
                    zraw = tmpA.tile([128, KC, TOK], BF16, tag="zraw", bufs=2)
                    nc.sync.dma_start(out=zraw[:], in_=d["zb"][:, :, cols])
                    craw = tmpA.tile([128, KC, TOK], BF16, tag="craw", bufs=2)
                    nc.sync.dma_start(out=craw[:], in_=d["cb"][:, :, cols])
                    znb = nrm.tile([128, KC, TOK], FP8, tag="znb", bufs=2)
                    cnb = nrm.tile([128, KC, TOK], FP8, tag="cnb", bufs=2)
                    norm_block(zraw, wcol["wz"], znb)
                    norm_block(craw, wcol["wc"], cnb)
                    kv_block(tb, znb, cnb)

            norm_scope.close()

            # ---------- phase B+C: attention (+ f1 interleaved) ----------
            with tc.tile_pool(name="wf", bufs=1) as wf, \
                 tc.tile_pool(name="fact", bufs=1) as fact, \
                 tc.tile_pool(name="relp", bufs=1) as relp, \
                 tc.tile_pool(name="att_t", bufs=1) as att_t, \
                 tc.tile_pool(name="ps_sc", bufs=2, space="PSUM") as ps_sc, \
                 tc.tile_pool(name="ps_av", bufs=1, space="PSUM") as ps_av, \
                 tc.tile_pool(name="ps_rs", bufs=1, space="PSUM") as ps_rs, \
                 tc.tile_pool(name="ps_f", bufs=1, space="PSUM") as ps_f:
                fw1 = wf.tile([128, 8, 2 * D], FP8, tag="fw1")
                nc.sync.dma_start(out=fw1[:], in_=d["fw18"][:])
                fw2 = wf.tile([128, 16, D], FP8, tag="fw2")
                nc.sync.dma_start(out=fw2[:], in_=d["fw28"][:])
                fh = fact.tile([128, 16, TOK], FP8, tag="fh")
                dzl8 = attn_sc.tile([128, KC, TOK], FP8, tag="dzl8")
                dzl16 = attn_sc.tile([128, KC, TOK], BF16, tag="dzl16")

                def f1_part(ob):
                    # one 256-wide output group of f1 (fp8 DR)
                    ps = ps_f.tile([128, 2, TOK], F32, tag="lin", bufs=1)
                    for m in range(2):
                        mc = ob * 2 + m
                        for i in range(4):
                            nc.tensor.matmul(
                                ps[:, m, :],
                                fw1[:, 2 * i:2 * i + 2,
                                    mc * 128:(mc + 1) * 128],
                                zn_own[:, 2 * i:2 * i + 2, :],
                                start=(i == 0), stop=(i == 3), perf_mode=DR)
                    for m in range(2):
                        mc = ob * 2 + m
                        evict_silu(fh[:, mc, :], ps[:, m, :],
                                   bias["fb1"][:, mc:mc + 1], fact)

                for c in range(KC):  # head pairs (2c, 2c+1)
                    rel = relp.tile([128, 16, 2, TOK], FP8, tag="rel", bufs=2)
                    for kc in range(16):
                        ps = ps_sc.tile([128, 2, TOK], F32, tag="sc")
                        nc.tensor.matmul(
                            ps[:, 0, :],
                            KT8[0:64, c, kc * 128:(kc + 1) * 128],
                            QT8[0:64, c, :], start=True, stop=True,
                            tile_position=(0, 0))
                        nc.tensor.matmul(
                            ps[:, 1, :],
                            KT8[64:128, c, kc * 128:(kc + 1) * 128],
                            QT8[64:128, c, :], start=True, stop=True,
                            tile_position=(64, 0))
                        nc.scalar.activation(rel[:, kc, :, :], ps[:],
                                             AF.Sigmoid, scale=sig_scale)
                    f1_part(c)
                    for par in range(2):  # head h = 2c + par
                        h = 2 * c + par
                        av = ps_av.tile([65, TOK], F32, tag="av")
                        for j in range(8):
                            nc.tensor.matmul(
                                av[:], V8[:, 2 * j:2 * j + 2, h, 0:65],
                                rel[:, 2 * j:2 * j + 2, par, :],
                                start=(j == 0), stop=(j == 7), perf_mode=DR)
                        rsf = att_t.tile([1, TOK], F32, tag="rsf", bufs=2)
                        nc.scalar.activation(rsf[:], av[64:65, :], AF.Copy)
                        bc = ps_rs.tile([64, TOK], F32, tag="bcrs")
                        nc.tensor.matmul(bc[:], ones_bc[0:1, 0:64], rsf[:],
                                         start=True, stop=True)
                        # rel_sum clip at 1.0, reciprocal, apply
                        mx = att_t.tile([64, TOK], F32, tag="mx", bufs=2)
                        nc.vector.tensor_scalar_max(mx[:], bc[:], 1.0)
                        nc.vector.reciprocal(mx[:], mx[:])
                        po = (h % 2) * 64
                        nc.vector.tensor_mul(attnT[po:po + 64, c, :],
                                             av[0:64, :], mx[:])

                # ---------- f2 (needs fh complete) ----------
                def f2_evict(mc, ps):
                    nc.vector.tensor_scalar_add(dzl16[:, mc, :], ps,
                                                bias["fb2"][:, mc:mc + 1])
                    nc.vector.tensor_copy(dzl8[:, mc, :], dzl16[:, mc, :])

                lin8(fw2, 2 * D, D, lambda i: fh[:, 2 * i:2 * i + 2, :],
                     f2_evict, ps_f, psbufs=1)

            # ---------- phase D: g MLP, o-proj, z1 ----------
            with tc.tile_pool(name="wg", bufs=1) as wg, \
                 tc.tile_pool(name="gact", bufs=1) as gact, \
                 tc.tile_pool(name="ps_d", bufs=2, space="PSUM") as ps_d:
                gw1 = wg.tile([128, 16, D], FP8, tag="gw1")
                nc.sync.dma_start(out=gw1[:], in_=d["gw18"][:])
                gw2 = wg.tile([128, 8, D], FP8, tag="gw2")
                nc.sync.dma_start(out=gw2[:], in_=d["gw28"][:])
                ow = wg.tile([128, 8, D], BF16, tag="ow")
                nc.sync.dma_start(out=ow[:], in_=d["owb"][:])
                zf = wg.tile([128, KC, TOK], F32, tag="zf")
                nc.sync.dma_start(out=zf[:], in_=d["zf"][:])

                gh = gact.tile([128, KC, TOK], FP8, tag="gh")
                s_f = gact.tile([128, KC, TOK], F32, tag="s_f")

                def g1_evict(mc, ps):
                    nc.scalar.activation(gh[:, mc, :], ps, AF.Tanh,
                                         bias=bias["gb1"][:, mc:mc + 1])

                lin8(gw1, 2 * D, D, rhs_cat(cn_own, dzl8), g1_evict, ps_d)

                def g2_evict(mc, ps):
                    nc.vector.scalar_tensor_tensor(
                        s_f[:, mc, :], ps, bias["gb2"][:, mc:mc + 1],
                        dzl16[:, mc, :], op0=mybir.AluOpType.add,
                        op1=mybir.AluOpType.add)
                    nc.vector.tensor_copy(s_b16[:, mc, :], s_f[:, mc, :])

                lin8(gw2, D, D, lambda i: gh[:, 2 * i:2 * i + 2, :], g2_evict,
                     ps_d)

                def o_evict(mc, ps):
                    t = gact.tile([128, TOK], F32, tag="t_z1", bufs=2)
                    nc.vector.scalar_tensor_tensor(
                        t[:], s_f[:, mc, :], dt_val, ps,
                        op0=mybir.AluOpType.mult, op1=mybir.AluOpType.add)
                    nc.vector.tensor_add(z1f[:, mc, :], t[:], zf[:, mc, :])
                    nc.vector.tensor_copy(z1b[:, mc, :], z1f[:, mc, :])

                lin16(ow, D, D, lambda k: attnT[:, k, :], o_evict, ps_d)

        # ---------- phase E: connection update ----------
        with tc.tile_pool(name="wcu", bufs=1) as wcu, \
             tc.tile_pool(name="cuact", bufs=1) as cuact, \
             tc.tile_pool(name="outp", bufs=1) as outp, \
             tc.tile_pool(name="ps_e", bufs=2, space="PSUM") as ps_e:
            cuw1 = wcu.tile([128, 24, 2 * D], BF16, tag="cuw1")
            nc.sync.dma_start(out=cuw1[:], in_=d["cuw1b"][:])
            c_raw = cuact.tile([128, KC, TOK], BF16, tag="c_raw")
            nc.sync.dma_start(out=c_raw[:], in_=d["cb"][:, :, 0:TOK])
            cf = cuact.tile([128, KC, TOK], F32, tag="cf")
            nc.sync.dma_start(out=cf[:], in_=d["cf"][:])
            du = cuact.tile([128, 16, TOK], BF16, tag="du")

            def cu1_rhs(k):
                if k < KC:
                    return c_raw[:, k, :]
                if k < 2 * KC:
                    return z1b[:, k - KC, :]
                return s_b16[:, k - 2 * KC, :]

            def cu1_evict(mc, ps):
                evict_silu(du[:, mc, :], ps, bias["cub1"][:, mc:mc + 1],
                           cuact)

            lin16(cuw1, 3 * D, 2 * D, cu1_rhs, cu1_evict, ps_e)

            with tc.tile_pool(name="wcu2", bufs=1) as wcu2:
                cuw2 = wcu2.tile([128, 16, D], BF16, tag="cuw2")
                nc.sync.dma_start(out=cuw2[:], in_=d["cuw2b"][:])

                def cu2_evict(mc, ps):
                    co = outp.tile([128, TOK], F32, tag="co")
                    nc.vector.scalar_tensor_tensor(
                        co[:], ps, bias["cub2"][:, mc:mc + 1], cf[:, mc, :],
                        op0=mybir.AluOpType.add, op1=mybir.AluOpType.add)
                    nc.sync.dma_start(out=cno_d[:, mc, :], in_=co[:])

                lin16(cuw2, 2 * D, D, lambda k: du[:, k, :], cu2_evict, ps_e)

        # ---------- phase F: final MLP ----------
        de_scope.close()
        with tc.tile_pool(name="wm", bufs=1) as wm, \
             tc.tile_pool(name="mact", bufs=1) as mact, \
             tc.tile_pool(name="outp2", bufs=2) as outp2, \
             tc.tile_pool(name="ps_ss2", bufs=2, space="PSUM") as ps_ss2, \
             tc.tile_pool(name="ps_bc2", bufs=2, space="PSUM") as ps_bc2, \
             tc.tile_pool(name="ps_m", bufs=2, space="PSUM") as ps_m:
            mw1 = wm.tile([128, 8, 4 * D], BF16, tag="mw1")
            nc.sync.dma_start(out=mw1[:], in_=d["mw1b"][:])
            z1n = mact.tile([128, KC, TOK], BF16, tag="z1n")

            # z1n = rms(z1) * wmlp
            ss = ps_ss2.tile([1, TOK], F32, tag="ss2")
            for k in range(KC):
                sq = mact.tile([128, TOK], BF16, tag="sq2", bufs=2)
                nc.vector.tensor_mul(sq[:], z1b[:, k, :], z1b[:, k, :])
                nc.tensor.matmul(ss[:], ones_col[:], sq[:],
                                 start=(k == 0), stop=(k == KC - 1))
            sf = mact.tile([1, TOK], F32, tag="sf2")
            nc.scalar.activation(sf[:], ss[:], AF.Sqrt, bias=eps1[:],
                                 scale=1.0 / D)
            bc = ps_bc2.tile([128, TOK], F32, tag="bc2")
            nc.tensor.matmul(bc[:], ones_bc[:], sf[:], start=True, stop=True)
            bcr = mact.tile([128, TOK], F32, tag="bcr2")
            nc.vector.reciprocal(bcr[:], bc[:])
            for k in range(KC):
                nc.vector.scalar_tensor_tensor(
                    z1n[:, k, :], bcr[:], wcol["wmlp"][:, k:k + 1],
                    z1b[:, k, :], op0=mybir.AluOpType.mult,
                    op1=mybir.AluOpType.mult)

            mh = mact.tile([128, 32, TOK], BF16, tag="mh")

            def m1_evict(mc, ps):
                evict_silu(mh[:, mc, :], ps, bias["mb1"][:, mc:mc + 1], mact)

            lin16(mw1, D, 4 * D, lambda k: z1n[:, k, :], m1_evict, ps_m)

            with tc.tile_pool(name="wm2", bufs=1) as wm2:
                mw2 = wm2.tile([128, 32, D], BF16, tag="mw2")
                nc.sync.dma_start(out=mw2[:], in_=d["mw2b"][:])

                def m2_evict(mc, ps):
                    zo = outp2.tile([128, TOK], F32, tag="zo")
                    nc.vector.scalar_tensor_tensor(
                        zo[:], ps, bias["mb2"][:, mc:mc + 1], z1f[:, mc, :],
                        op0=mybir.AluOpType.add, op1=mybir.AluOpType.add)
                    nc.sync.dma_start(out=z2o_d[:, mc, :], in_=zo[:])

                lin16(mw2, 4 * D, D, lambda k: mh[:, k, :], m2_evict, ps_m)


_CACHE = {}
_RUN_KWARGS = {}
_LAST_RESULT = None

_E4 = ml_dtypes.float8_e4m3
_BF = ml_dtypes.bfloat16


def _tile3(wT, dtp):
    # [K, M] -> [128, K//128, M] contiguous
    K, M = wT.shape
    return np.ascontiguousarray(
        np.asarray(wT, np.float32).reshape(K // 128, 128, M)
        .transpose(1, 0, 2)).astype(dtp)


def _prep_shared(inputs):
    dt_val = float(np.asarray(inputs["dt"]))
    cu1 = np.asarray(inputs["cu_w1"], np.float32).copy()
    cu1[:, 2 * D:] *= dt_val  # fold dz = dt*s into cu_w1's dz block
    t = lambda x: np.asarray(x, np.float32).T
    shared = {
        "qw8": _tile3(t(inputs["q_w"]), _E4),
        "kw8": _tile3(t(inputs["k_w"]), _E4),
        "vw8": _tile3(t(inputs["v_w"]), _E4),
        "fw18": _tile3(t(inputs["f_w1"]), _E4),
        "fw28": _tile3(t(inputs["f_w2"]), _E4),
        "gw18": _tile3(t(inputs["g_w1"]), _E4),
        "gw28": _tile3(t(inputs["g_w2"]), _E4),
        "owb": _tile3(t(inputs["o_w"]), _BF),
        "cuw1b": _tile3(cu1.T, _BF),
        "cuw2b": _tile3(t(inputs["cu_w2"]), _BF),
        "mw1b": _tile3(t(inputs["m_w1"]), _BF),
        "mw2b": _tile3(t(inputs["m_w2"]), _BF),
    }
    for name, key in [("fb1", "f_b1"), ("fb2", "f_b2"), ("gb1", "g_b1"),
                      ("gb2", "g_b2"), ("cub1", "cu_b1"), ("cub2", "cu_b2"),
                      ("mb1", "m_b1"), ("mb2", "m_b2"), ("wz", "w_z"),
                      ("wc", "w_c"), ("wmlp", "w_mlp")]:
        shared[name] = np.ascontiguousarray(np.asarray(inputs[key], np.float32))
    return shared


def _core_maps(inputs, shared):
    z = np.asarray(inputs["z"], np.float32)
    conn = np.asarray(inputs["connection"], np.float32)
    zT = [np.ascontiguousarray(z[b].T) for b in range(B)]
    cT = [np.ascontiguousarray(conn[b].T) for b in range(B)]
    in_maps = []
    for c in range(NCORES):
        b, tb = divmod(c, NTB)
        zr = np.roll(zT[b], -tb * TOK, axis=1)
        cr = np.roll(cT[b], -tb * TOK, axis=1)
        m = dict(shared)
        m["zb"] = _tile3(zr, _BF)
        m["cb"] = _tile3(cr, _BF)
        m["zf"] = _tile3(zr[:, 0:TOK], np.float32)
        m["cf"] = _tile3(cr[:, 0:TOK], np.float32)
        in_maps.append(m)
    return in_maps


def kernel(**inputs):
    z = np.asarray(inputs["z"], np.float32)
    dt_val = float(np.asarray(inputs["dt"]))
    temp_val = float(np.asarray(inputs["temp"]))

    key = (dt_val, temp_val)
    if key not in _CACHE:
        _CACHE[key] = build_program(dt_val, temp_val)
    nc = _CACHE[key]

    in_maps = _core_maps(inputs, _prep_shared(inputs))
    res = run_bass_kernel_spmd(nc, in_maps, list(range(NCORES)), **_RUN_KWARGS)
    global _LAST_RESULT
    _LAST_RESULT = res

    z2 = np.empty((B, L, D), np.float32)
    conn_new = np.empty((B, L, D), np.float32)
    for c in range(NCORES):
        b, tb = divmod(c, NTB)
        sl = slice(tb * TOK, (tb + 1) * TOK)
        z2[b, sl, :] = res.results[c]["z2o"].transpose(1, 0, 2).reshape(D, TOK).T
        conn_new[b, sl, :] = res.results[c]["cno"].transpose(1, 0, 2).reshape(D, TOK).T
    return z2, conn_new, z


# revision 42
# speedup vs baseline: 1.7090x; 1.0729x over previous
"""CovariantEvolutionBlock Trainium2 kernel (v2).

Strategy: token-parallel over B*L across 8 cores (512 own tokens/core), zero
collectives; inputs rotated per-core so own tokens are keys block 0. All
activations kept feature-major [128, chunk, tokens] on-chip.

v2 changes vs v1 (1.33ms baseline):
- Weights are host-pre-tiled to [128, K/128, M] and DMA'd exactly once in a
  few large contiguous transfers (v1 streamed 64KB tiles, re-reading K/V
  weights 4x: 133MB -> ~62MB HBM traffic).
- fp8 (e4m3) + DoubleRow matmuls for Q/K/V/scores/attnV/f/g paths (halves
  matmul instruction count there; verified 3.5e-3 end-to-end rel err in
  numpy vs the 2e-2 budget). o/cu/m paths stay bf16 (fp8 there breaks the
  error budget since delta/mlp feed residuals undamped).
- Attention scores packed two heads per PE pass via tile_position row
  packing (contraction is only 64 wide per head).
- Norm/rel_sum chains avoid 1-partition vector ops and gpsimd broadcasts:
  scalar copy/sqrt -> K=1 fp32 matmul broadcast -> vector reciprocal on
  [64|128, 512] tiles.
- f1 MLP interleaved under attention so the 256 sigmoid evictions (scalar
  engine) overlap tensor work.
"""

import sys

try:
    import concourse.bass as bass  # noqa: F401
except ImportError:
    sys.path.insert(0, "/opt/trn_rl_repo")

import numpy as np
import ml_dtypes

import concourse.bacc as bacc
import concourse.tile as tile
import concourse.mybir as mybir
from concourse.bass_utils import run_bass_kernel_spmd

F32 = mybir.dt.float32
BF16 = mybir.dt.bfloat16
FP8 = mybir.dt.float8e4
AF = mybir.ActivationFunctionType
DR = mybir.MatmulPerfMode.DoubleRow

B, L, D, H, HD = 2, 2048, 1024, 16, 64
EPS = 1e-6
NCORES = 8
TOK = 512          # own tokens per core
KEYS = 2048        # keys per batch
KC = D // 128      # 8 feature chunks of 128
NTB = KEYS // TOK  # 4 token blocks per batch


def _bias_ap(dram_ap):
    # [dim] -> [128, dim//128]: tile[p, c] = bias[c*128 + p]
    return dram_ap.rearrange("(c p) -> p c", p=128)


def build_program(dt_val: float, temp_val: float):
    nc = bacc.Bacc("TRN2", target_bir_lowering=False, debug=False,
                   num_devices=NCORES)

    d = {}
    for name, shape, dtp in [
        ("zb", [128, KC, KEYS], BF16), ("cb", [128, KC, KEYS], BF16),
        ("zf", [128, KC, TOK], F32), ("cf", [128, KC, TOK], F32),
        ("qw8", [128, 16, D], FP8), ("kw8", [128, 16, D], FP8),
        ("vw8", [128, 8, D], FP8),
        ("fw18", [128, 8, 2 * D], FP8), ("fw28", [128, 16, D], FP8),
        ("gw18", [128, 16, D], FP8), ("gw28", [128, 8, D], FP8),
        ("owb", [128, 8, D], BF16),
        ("cuw1b", [128, 24, 2 * D], BF16), ("cuw2b", [128, 16, D], BF16),
        ("mw1b", [128, 8, 4 * D], BF16), ("mw2b", [128, 32, D], BF16),
        ("fb1", [2 * D], F32), ("fb2", [D], F32),
        ("gb1", [D], F32), ("gb2", [D], F32),
        ("cub1", [2 * D], F32), ("cub2", [D], F32),
        ("mb1", [4 * D], F32), ("mb2", [D], F32),
        ("wz", [D], F32), ("wc", [D], F32), ("wmlp", [D], F32),
    ]:
        d[name] = nc.dram_tensor(name, shape, dtp, kind="ExternalInput").ap()

    z2o_d = nc.dram_tensor("z2o", [128, KC, TOK], F32, kind="ExternalOutput").ap()
    cno_d = nc.dram_tensor("cno", [128, KC, TOK], F32, kind="ExternalOutput").ap()

    sig_scale = float(temp_val) * (HD ** -0.5)

    with tile.TileContext(nc) as tc:
        _emit(nc, tc, d, z2o_d, cno_d, float(dt_val), sig_scale)
    nc.compile()
    return nc


def _emit(nc, tc, d, z2o_d, cno_d, dt_val, sig_scale):
    from contextlib import ExitStack

    ctx = ExitStack()
    with ctx:
        # ---------- pools alive the whole program ----------
        const = ctx.enter_context(tc.tile_pool(name="const", bufs=1))
        persist = ctx.enter_context(tc.tile_pool(name="persist", bufs=1))

        # tiles only; DMAs are emitted inside phase A after the first
        # activation loads so the sync queue serves the critical path first
        bias = {}
        for name in ["fb1", "fb2", "gb1", "gb2", "cub1", "cub2", "mb1", "mb2"]:
            n = d[name].shape[0]
            bt = const.tile([128, n // 128], F32, tag=name)
            bias[name] = bt
        wcol = {}
        for name in ["wz", "wc", "wmlp"]:
            wt_ = const.tile([128, KC], F32, tag=name)
            wcol[name] = wt_

        def load_consts(names):
            for name in names:
                src = bias[name] if name in bias else wcol[name]
                nc.sync.dma_start(out=src[:], in_=_bias_ap(d[name]))

        ones_col = const.tile([128, 1], BF16, tag="ones_col")
        nc.vector.memset(ones_col[:], 1.0)
        ones_bc = const.tile([1, 128], F32, tag="ones_bc")
        nc.vector.memset(ones_bc[:], 1.0)
        eps1 = const.tile([1, 1], F32, tag="eps1")
        nc.vector.memset(eps1[:], EPS)

        # persistent across phase boundaries (outlive the attn scope)
        z1b = persist.tile([128, KC, TOK], BF16, tag="z1b")
        z1f = persist.tile([128, KC, TOK], F32, tag="z1f")
        # s_b16 spans phases D..E only; its pool closes before the final MLP
        de_scope = ctx.enter_context(ExitStack())
        de_pool = de_scope.enter_context(tc.tile_pool(name="de", bufs=1))
        s_b16 = de_pool.tile([128, KC, TOK], BF16, tag="s_b16")

        # ---------- helpers ----------
        def lin8(wt, n_in, n_out, rhs_fn, evict_fn, pool, psbufs=2):
            # fp8 DoubleRow linear: out[mc] = sum_i wt[:,2i:2i+2,mc].T2 @ rhs(i)
            ndr = n_in // 256
            for ob in range(n_out // 256):
                ps = pool.tile([128, 2, TOK], F32, tag="lin", bufs=psbufs)
                for m in range(2):
                    mc = ob * 2 + m
                    for i in range(ndr):
                        nc.tensor.matmul(
                            ps[:, m, :],
                            wt[:, 2 * i:2 * i + 2, mc * 128:(mc + 1) * 128],
                            rhs_fn(i), start=(i == 0), stop=(i == ndr - 1),
                            perf_mode=DR)
                for m in range(2):
                    evict_fn(ob * 2 + m, ps[:, m, :])

        def lin16(wt, n_in, n_out, rhs_fn, evict_fn, pool, psbufs=2):
            # bf16 linear, weights resident in SBUF
            kcn = n_in // 128
            for ob in range(n_out // 256):
                ps = pool.tile([128, 2, TOK], F32, tag="lin", bufs=psbufs)
                for m in range(2):
                    mc = ob * 2 + m
                    for k in range(kcn):
                        nc.tensor.matmul(
                            ps[:, m, :], wt[:, k, mc * 128:(mc + 1) * 128],
                            rhs_fn(k), start=(k == 0), stop=(k == kcn - 1))
                for m in range(2):
                    evict_fn(ob * 2 + m, ps[:, m, :])

        def evict_silu(dst, ps, bias_ap, sg_pool):
            sg = sg_pool.tile([128, TOK], BF16, tag="sg", bufs=2)
            nc.scalar.activation(sg[:], ps, AF.Sigmoid, bias=bias_ap)
            nc.vector.scalar_tensor_tensor(
                dst, ps, bias_ap, sg[:],
                op0=mybir.AluOpType.add, op1=mybir.AluOpType.mult)

        # ---------- phase A: norms, Q, K, V ----------
        with tc.tile_pool(name="attn_sc", bufs=1) as attn_sc:
            zn_own = attn_sc.tile([128, KC, TOK], FP8, tag="zn_own")
            cn_own = attn_sc.tile([128, KC, TOK], FP8, tag="cn_own")
            attnT = attn_sc.tile([128, KC, TOK], BF16, tag="attnT")
            dzl8 = attn_sc.tile([128, KC, TOK], FP8, tag="dzl8")
            dzl16 = attn_sc.tile([128, KC, TOK], BF16, tag="dzl16")

            kvq_scope = ExitStack()
            kvq_sc = kvq_scope.enter_context(
                tc.tile_pool(name="kvq_sc", bufs=1))
            KT8 = kvq_sc.tile([128, KC, KEYS], FP8, tag="KT8")
            V8 = kvq_sc.tile([128, 16, H, 66], FP8, tag="V8")
            QT8 = kvq_sc.tile([128, KC, TOK], FP8, tag="QT8")
            nc.vector.memset(V8[:, :, :, 64:65], 1.0)

            norm_scope = ExitStack()
            nrm = norm_scope.enter_context(tc.tile_pool(name="nrm", bufs=1))
            tmpA = norm_scope.enter_context(tc.tile_pool(name="tmpA", bufs=1))
            ps_ss = norm_scope.enter_context(
                tc.tile_pool(name="ps_ss", bufs=2, space="PSUM"))
            ps_bc = norm_scope.enter_context(
                tc.tile_pool(name="ps_bc", bufs=2, space="PSUM"))
            ps_A = norm_scope.enter_context(
                tc.tile_pool(name="ps_A", bufs=1, space="PSUM"))

            def norm_block(xraw, w_t, dst, dst_dt_hint=None):
                # dst[:, k, :] = xraw[k] * rsqrt(mean(x^2) + eps) * w
                ss = ps_ss.tile([1, TOK], F32, tag="ss")
                for k in range(KC):
                    sq = nrm.tile([128, TOK], BF16, tag="sq", bufs=2)
                    nc.vector.tensor_mul(sq[:], xraw[:, k, :], xraw[:, k, :])
                    nc.tensor.matmul(ss[:], ones_col[:], sq[:],
                                     start=(k == 0), stop=(k == KC - 1))
                sf = nrm.tile([1, TOK], F32, tag="sf", bufs=2)
                nc.scalar.activation(sf[:], ss[:], AF.Sqrt,
                                     bias=eps1[:], scale=1.0 / D)
                bc = ps_bc.tile([128, TOK], F32, tag="bc")
                nc.tensor.matmul(bc[:], ones_bc[:], sf[:], start=True,
                                 stop=True)
                bcr = nrm.tile([128, TOK], F32, tag="bcr", bufs=2)
                nc.vector.reciprocal_approx_fast(out=bcr[:], in_=bc[:])
                for k in range(KC):
                    nc.vector.scalar_tensor_tensor(
                        dst[:, k, :], bcr[:], w_t[:, k:k + 1], xraw[:, k, :],
                        op0=mybir.AluOpType.mult, op1=mybir.AluOpType.mult)

            def rhs_cat(zt, ct):
                def f(i):
                    if i < 4:
                        return zt[:, 2 * i:2 * i + 2, :]
                    return ct[:, 2 * (i - 4):2 * (i - 4) + 2, :]
                return f

            # --- own block (tb=0): norms + Q ---
            zraw0 = tmpA.tile([128, KC, TOK], BF16, tag="zraw", bufs=2)
            nc.sync.dma_start(out=zraw0[:], in_=d["zb"][:, :, 0:TOK])
            craw0 = tmpA.tile([128, KC, TOK], BF16, tag="craw", bufs=2)
            nc.sync.dma_start(out=craw0[:], in_=d["cb"][:, :, 0:TOK])
            load_consts(["wz", "wc"])
            norm_block(zraw0, wcol["wz"], zn_own)
            norm_block(craw0, wcol["wc"], cn_own)

            with tc.tile_pool(name="wq", bufs=1) as wq:
                qw = wq.tile([128, 16, D], FP8, tag="qw")
                nc.sync.dma_start(out=qw[:], in_=d["qw8"][:])
                load_consts(["fb1", "fb2", "gb1", "gb2", "cub1", "cub2",
                             "mb1", "mb2", "wmlp"])

                def q_evict(mc, ps):
                    nc.vector.tensor_copy(QT8[:, mc, :], ps)

                lin8(qw, 2 * D, D, rhs_cat(zn_own, cn_own), q_evict, ps_A)

            # --- K/V for all 4 blocks ---
            with tc.tile_pool(name="wkv", bufs=1) as wkv:
                kw = wkv.tile([128, 16, D], FP8, tag="kw")
                nc.sync.dma_start(out=kw[:], in_=d["kw8"][:])
                vw = wkv.tile([128, 8, D], FP8, tag="vw")
                nc.sync.dma_start(out=vw[:], in_=d["vw8"][:])

                def kv_block(tb, zt, ct):
                    cols = slice(tb * TOK, (tb + 1) * TOK)
                    # V: stationary = zn tiles, moving = vw -> keys-major out
                    for kt in range(4):
                        kcg = tb * 4 + kt
                        ps = ps_A.tile([128, 2, TOK], F32, tag="lin", bufs=2)
                        for vb in range(2):
                            for i in range(4):
                                nc.tensor.matmul(
                                    ps[:, vb, :],
                                    zt[:, 2 * i:2 * i + 2,
                                       kt * 128:(kt + 1) * 128],
                                    vw[:, 2 * i:2 * i + 2,
                                       vb * TOK:(vb + 1) * TOK],
                                    start=(i == 0), stop=(i == 3),
                                    perf_mode=DR)
                        for vb in range(2):
                            src = ps[:, vb, :].rearrange(
                                "p (h d) -> p h d", h=8)
                            nc.vector.tensor_copy(
                                V8[:, kcg, vb * 8:(vb + 1) * 8, 0:HD], src)

                    def k_evict(mc, ps):
                        nc.vector.tensor_copy(KT8[:, mc, cols], ps)

                    lin8(kw, 2 * D, D, rhs_cat(zt, ct), k_evict, ps_A)

                kv_block(0, zn_own, cn_own)
                for tb in range(1, NTB):
                    cols = slice(tb * TOK, (tb + 1) * TOK)
                    zraw = tmpA.tile([128, KC, TOK], BF16, tag="zraw", bufs=2)
                    nc.sync.dma_start(out=zraw[:], in_=d["zb"][:, :, cols])
                    craw = tmpA.tile([128, KC, TOK], BF16, tag="craw", bufs=2)
                    nc.sync.dma_start(out=craw[:], in_=d["cb"][:, :, cols])
                    znb = nrm.tile([128, KC, TOK], FP8, tag="znb", bufs=2)
                    cnb = nrm.tile([128, KC, TOK], FP8, tag="cnb", bufs=2)
                    norm_block(zraw, wcol["wz"], znb)
                    norm_block(craw, wcol["wc"], cnb)
                    kv_block(tb, znb, cnb)

            norm_scope.close()

            # ---------- phase B+C: attention (+ f1 interleaved) ----------
            with tc.tile_pool(name="wf", bufs=1) as wf, \
                 tc.tile_pool(name="fact", bufs=1) as fact, \
                 tc.tile_pool(name="relp", bufs=1) as relp, \
                 tc.tile_pool(name="att_t", bufs=1) as att_t, \
                 tc.tile_pool(name="ps_sc", bufs=2, space="PSUM") as ps_sc, \
                 tc.tile_pool(name="ps_av", bufs=1, space="PSUM") as ps_av, \
                 tc.tile_pool(name="ps_rs", bufs=1, space="PSUM") as ps_rs, \
                 tc.tile_pool(name="ps_f", bufs=1, space="PSUM") as ps_f:
                fw1 = wf.tile([128, 8, 2 * D], FP8, tag="fw1")
                nc.sync.dma_start(out=fw1[:], in_=d["fw18"][:])
                fw2 = wf.tile([128, 16, D], FP8, tag="fw2")
                nc.sync.dma_start(out=fw2[:], in_=d["fw28"][:])
                fh = fact.tile([128, 16, TOK], FP8, tag="fh")

                def f1_part(ob):
                    # one 256-wide output group of f1 (fp8 DR)
                    ps = ps_f.tile([128, 2, TOK], F32, tag="lin", bufs=1)
                    for m in range(2):
                        mc = ob * 2 + m
                        for i in range(4):
                            nc.tensor.matmul(
                                ps[:, m, :],
                                fw1[:, 2 * i:2 * i + 2,
                                    mc * 128:(mc + 1) * 128],
                                zn_own[:, 2 * i:2 * i + 2, :],
                                start=(i == 0), stop=(i == 3), perf_mode=DR)
                    for m in range(2):
                        mc = ob * 2 + m
                        evict_silu(fh[:, mc, :], ps[:, m, :],
                                   bias["fb1"][:, mc:mc + 1], fact)

                def f2_part(ob):
                    ps = ps_f.tile([128, 2, TOK], F32, tag="lin", bufs=1)
                    for m in range(2):
                        mc = ob * 2 + m
                        for i in range(8):
                            nc.tensor.matmul(
                                ps[:, m, :],
                                fw2[:, 2 * i:2 * i + 2,
                                    mc * 128:(mc + 1) * 128],
                                fh[:, 2 * i:2 * i + 2, :],
                                start=(i == 0), stop=(i == 7), perf_mode=DR)
                    for m in range(2):
                        mc = ob * 2 + m
                        nc.vector.tensor_scalar_add(dzl16[:, mc, :],
                                                    ps[:, m, :],
                                                    bias["fb2"][:, mc:mc + 1])
                        nc.vector.tensor_copy(dzl8[:, mc, :], dzl16[:, mc, :])

                for c in range(KC):  # head pairs (2c, 2c+1)
                    rel = relp.tile([128, 16, 2, TOK], FP8, tag="rel", bufs=2)
                    for kc in range(16):
                        ps = ps_sc.tile([128, 2, TOK], F32, tag="sc")
                        nc.tensor.matmul(
                            ps[:, 0, :],
                            KT8[0:64, c, kc * 128:(kc + 1) * 128],
                            QT8[0:64, c, :], start=True, stop=True,
                            tile_position=(0, 0))
                        nc.tensor.matmul(
                            ps[:, 1, :],
                            KT8[64:128, c, kc * 128:(kc + 1) * 128],
                            QT8[64:128, c, :], start=True, stop=True,
                            tile_position=(64, 0))
                        nc.scalar.activation(rel[:, kc, :, :], ps[:],
                                             AF.Sigmoid, scale=sig_scale)
                    if c < 4:
                        f1_part(2 * c)
                        f1_part(2 * c + 1)
                    else:
                        f2_part(c - 4)
                    for par in range(2):  # head h = 2c + par
                        h = 2 * c + par
                        av = ps_av.tile([65, TOK], F32, tag="av")
                        for j in range(8):
                            nc.tensor.matmul(
                                av[:], V8[:, 2 * j:2 * j + 2, h, 0:65],
                                rel[:, 2 * j:2 * j + 2, par, :],
                                start=(j == 0), stop=(j == 7), perf_mode=DR)
                        rsf = att_t.tile([1, TOK], F32, tag="rsf", bufs=2)
                        nc.scalar.activation(rsf[:], av[64:65, :], AF.Copy)
                        bc = ps_rs.tile([64, TOK], F32, tag="bcrs")
                        nc.tensor.matmul(bc[:], ones_bc[0:1, 0:64], rsf[:],
                                         start=True, stop=True)
                        # rel_sum clip at 1.0, reciprocal, apply
                        mx = att_t.tile([64, TOK], F32, tag="mx", bufs=2)
                        nc.vector.tensor_scalar_max(mx[:], bc[:], 1.0)
                        nc.vector.reciprocal_approx_fast(out=mx[:], in_=mx[:])
                        po = (h % 2) * 64
                        nc.vector.tensor_mul(attnT[po:po + 64, c, :],
                                             av[0:64, :], mx[:])

            # kvq scope closed: KT/V/QT space is free; stage the first two
            # cu_w1 quarters there (right-side pool) so their DMAs overlap
            # all of phase D.
            kvq_scope.close()
            cu_scope = ctx.enter_context(ExitStack())
            wcu_r = cu_scope.enter_context(
                tc.tile_pool(name="wcu_r", bufs=1, side="right"))
            cu_tiles = []
            for q in range(2):
                t = wcu_r.tile([128, 24, TOK], BF16, tag="cuw1", bufs=2)
                nc.sync.dma_start(
                    out=t[:], in_=d["cuw1b"][:, :, q * TOK:(q + 1) * TOK])
                cu_tiles.append(t)

            # ---------- phase D: g MLP, o-proj, z1 ----------
            with tc.tile_pool(name="wg", bufs=1) as wg, \
                 tc.tile_pool(name="gact", bufs=1) as gact, \
                 tc.tile_pool(name="ps_d", bufs=2, space="PSUM") as ps_d:
                gw1 = wg.tile([128, 16, D], FP8, tag="gw1")
                nc.sync.dma_start(out=gw1[:], in_=d["gw18"][:])
                gw2 = wg.tile([128, 8, D], FP8, tag="gw2")
                nc.sync.dma_start(out=gw2[:], in_=d["gw28"][:])
                ow = wg.tile([128, 8, D], BF16, tag="ow")
                nc.sync.dma_start(out=ow[:], in_=d["owb"][:])
                zf = wg.tile([128, KC, TOK], F32, tag="zf")
                nc.sync.dma_start(out=zf[:], in_=d["zf"][:])

                gh = gact.tile([128, KC, TOK], FP8, tag="gh")
                s_f = gact.tile([128, KC, TOK], F32, tag="s_f")

                def g1_evict(mc, ps):
                    nc.scalar.activation(gh[:, mc, :], ps, AF.Tanh,
                                         bias=bias["gb1"][:, mc:mc + 1])

                lin8(gw1, 2 * D, D, rhs_cat(cn_own, dzl8), g1_evict, ps_d)

                def g2_evict(mc, ps):
                    nc.vector.scalar_tensor_tensor(
                        s_f[:, mc, :], ps, bias["gb2"][:, mc:mc + 1],
                        dzl16[:, mc, :], op0=mybir.AluOpType.add,
                        op1=mybir.AluOpType.add)
                    nc.vector.tensor_copy(s_b16[:, mc, :], s_f[:, mc, :])

                lin8(gw2, D, D, lambda i: gh[:, 2 * i:2 * i + 2, :], g2_evict,
                     ps_d)

                def o_evict(mc, ps):
                    t = gact.tile([128, TOK], F32, tag="t_z1", bufs=2)
                    nc.vector.scalar_tensor_tensor(
                        t[:], s_f[:, mc, :], dt_val, ps,
                        op0=mybir.AluOpType.mult, op1=mybir.AluOpType.add)
                    nc.vector.tensor_add(z1f[:, mc, :], t[:], zf[:, mc, :])
                    nc.vector.tensor_copy(z1b[:, mc, :], z1f[:, mc, :])

                lin16(ow, D, D, lambda k: attnT[:, k, :], o_evict, ps_d)

        # ---------- phase E: connection update ----------
        with tc.tile_pool(name="cuact", bufs=1) as cuact, \
             tc.tile_pool(name="wcu2", bufs=1) as wcu2, \
             tc.tile_pool(name="outp", bufs=1) as outp, \
             tc.tile_pool(name="ps_e", bufs=2, space="PSUM") as ps_e:
            cuw2 = wcu2.tile([128, 16, D], BF16, tag="cuw2")
            nc.sync.dma_start(out=cuw2[:], in_=d["cuw2b"][:])
            c_raw = cuact.tile([128, KC, TOK], BF16, tag="c_raw")
            nc.sync.dma_start(out=c_raw[:], in_=d["cb"][:, :, 0:TOK])
            cf = cuact.tile([128, KC, TOK], F32, tag="cf")
            nc.sync.dma_start(out=cf[:], in_=d["cf"][:])
            du = cuact.tile([128, 16, TOK], BF16, tag="du")

            def cu1_rhs(k):
                if k < KC:
                    return c_raw[:, k, :]
                if k < 2 * KC:
                    return z1b[:, k - KC, :]
                return s_b16[:, k - 2 * KC, :]

            # cu1 in four 512-wide output quarters; quarters 2-3 DMA during
            # compute of earlier quarters (bufs=2 rotation)
            for q in range(4):
                if q < 2:
                    wt = cu_tiles[q]
                else:
                    wt = wcu_r.tile([128, 24, TOK], BF16, tag="cuw1", bufs=2)
                    nc.sync.dma_start(
                        out=wt[:], in_=d["cuw1b"][:, :, q * TOK:(q + 1) * TOK])
                for ob in range(2):
                    ps = ps_e.tile([128, 2, TOK], F32, tag="lin")
                    for m in range(2):
                        lcol = (ob * 2 + m) * 128
                        for k in range(24):
                            nc.tensor.matmul(
                                ps[:, m, :], wt[:, k, lcol:lcol + 128],
                                cu1_rhs(k), start=(k == 0), stop=(k == 23))
                    for m in range(2):
                        mc = q * 4 + ob * 2 + m
                        evict_silu(du[:, mc, :], ps[:, m, :],
                                   bias["cub1"][:, mc:mc + 1], cuact)

            def cu2_evict(mc, ps):
                co = outp.tile([128, TOK], F32, tag="co")
                nc.vector.scalar_tensor_tensor(
                    co[:], ps, bias["cub2"][:, mc:mc + 1], cf[:, mc, :],
                    op0=mybir.AluOpType.add, op1=mybir.AluOpType.add)
                nc.sync.dma_start(out=cno_d[:, mc, :], in_=co[:])

            lin16(cuw2, 2 * D, D, lambda k: du[:, k, :], cu2_evict, ps_e)

        # ---------- phase F: final MLP ----------
        cu_scope.close()
        de_scope.close()
        with tc.tile_pool(name="wm", bufs=1, side="right") as wm, \
             tc.tile_pool(name="mact", bufs=1) as mact, \
             tc.tile_pool(name="outp2", bufs=2) as outp2, \
             tc.tile_pool(name="ps_ss2", bufs=2, space="PSUM") as ps_ss2, \
             tc.tile_pool(name="ps_bc2", bufs=2, space="PSUM") as ps_bc2, \
             tc.tile_pool(name="ps_m", bufs=2, space="PSUM") as ps_m:
            z1n = mact.tile([128, KC, TOK], BF16, tag="z1n")

            # z1n = rms(z1) * wmlp
            ss = ps_ss2.tile([1, TOK], F32, tag="ss2")
            for k in range(KC):
                sq = mact.tile([128, TOK], BF16, tag="sq2", bufs=2)
                nc.vector.tensor_mul(sq[:], z1b[:, k, :], z1b[:, k, :])
                nc.tensor.matmul(ss[:], ones_col[:], sq[:],
                                 start=(k == 0), stop=(k == KC - 1))
            sf = mact.tile([1, TOK], F32, tag="sf2")
            nc.scalar.activation(sf[:], ss[:], AF.Sqrt, bias=eps1[:],
                                 scale=1.0 / D)
            bc = ps_bc2.tile([128, TOK], F32, tag="bc2")
            nc.tensor.matmul(bc[:], ones_bc[:], sf[:], start=True, stop=True)
            bcr = mact.tile([128, TOK], F32, tag="bcr2")
            nc.vector.reciprocal_approx_fast(out=bcr[:], in_=bc[:])
            for k in range(KC):
                nc.vector.scalar_tensor_tensor(
                    z1n[:, k, :], bcr[:], wcol["wmlp"][:, k:k + 1],
                    z1b[:, k, :], op0=mybir.AluOpType.mult,
                    op1=mybir.AluOpType.mult)

            mh = mact.tile([128, 32, TOK], BF16, tag="mh")

            # m1 in two 2048-wide output halves (weight DMA overlaps compute)
            for hf in range(2):
                wt = wm.tile([128, 8, 2 * D], BF16, tag="mw1", bufs=2)
                nc.sync.dma_start(
                    out=wt[:], in_=d["mw1b"][:, :, hf * 2 * D:(hf + 1) * 2 * D])
                for ob in range(8):
                    ps = ps_m.tile([128, 2, TOK], F32, tag="lin")
                    for m in range(2):
                        lcol = (ob * 2 + m) * 128
                        for k in range(KC):
                            nc.tensor.matmul(
                                ps[:, m, :], wt[:, k, lcol:lcol + 128],
                                z1n[:, k, :], start=(k == 0),
                                stop=(k == KC - 1))
                    for m in range(2):
                        mc = hf * 16 + ob * 2 + m
                        evict_silu(mh[:, mc, :], ps[:, m, :],
                                   bias["mb1"][:, mc:mc + 1], mact)

            # m2 in two 512-wide output halves (4 chunks each)
            for hf in range(2):
                wt = wm.tile([128, 32, TOK], BF16, tag="mw2", bufs=2)
                nc.sync.dma_start(
                    out=wt[:], in_=d["mw2b"][:, :, hf * TOK:(hf + 1) * TOK])
                for ob in range(2):
                    ps = ps_m.tile([128, 2, TOK], F32, tag="lin")
                    for m in range(2):
                        lcol = (ob * 2 + m) * 128
                        for k in range(32):
                            nc.tensor.matmul(
                                ps[:, m, :], wt[:, k, lcol:lcol + 128],
                                mh[:, k, :], start=(k == 0), stop=(k == 31))
                    for m in range(2):
                        mc = hf * 4 + ob * 2 + m
                        zo = outp2.tile([128, TOK], F32, tag="zo")
                        nc.vector.scalar_tensor_tensor(
                            zo[:], ps[:, m, :], bias["mb2"][:, mc:mc + 1],
                            z1f[:, mc, :], op0=mybir.AluOpType.add,
                            op1=mybir.AluOpType.add)
                        nc.sync.dma_start(out=z2o_d[:, mc, :], in_=zo[:])


_CACHE = {}
_RUN_KWARGS = {}
_LAST_RESULT = None

_E4 = ml_dtypes.float8_e4m3
_BF = ml_dtypes.bfloat16


def _tile3(wT, dtp):
    # [K, M] -> [128, K//128, M] contiguous
    K, M = wT.shape
    return np.ascontiguousarray(
        np.asarray(wT, np.float32).reshape(K // 128, 128, M)
        .transpose(1, 0, 2)).astype(dtp)


def _prep_shared(inputs):
    dt_val = float(np.asarray(inputs["dt"]))
    cu1 = np.asarray(inputs["cu_w1"], np.float32).copy()
    cu1[:, 2 * D:] *= dt_val  # fold dz = dt*s into cu_w1's dz block
    t = lambda x: np.asarray(x, np.float32).T
    shared = {
        "qw8": _tile3(t(inputs["q_w"]), _E4),
        "kw8": _tile3(t(inputs["k_w"]), _E4),
        "vw8": _tile3(t(inputs["v_w"]), _E4),
        "fw18": _tile3(t(inputs["f_w1"]), _E4),
        "fw28": _tile3(t(inputs["f_w2"]), _E4),
        "gw18": _tile3(t(inputs["g_w1"]), _E4),
        "gw28": _tile3(t(inputs["g_w2"]), _E4),
        "owb": _tile3(t(inputs["o_w"]), _BF),
        "cuw1b": _tile3(cu1.T, _BF),
        "cuw2b": _tile3(t(inputs["cu_w2"]), _BF),
        "mw1b": _tile3(t(inputs["m_w1"]), _BF),
        "mw2b": _tile3(t(inputs["m_w2"]), _BF),
    }
    for name, key in [("fb1", "f_b1"), ("fb2", "f_b2"), ("gb1", "g_b1"),
                      ("gb2", "g_b2"), ("cub1", "cu_b1"), ("cub2", "cu_b2"),
                      ("mb1", "m_b1"), ("mb2", "m_b2"), ("wz", "w_z"),
                      ("wc", "w_c"), ("wmlp", "w_mlp")]:
        shared[name] = np.ascontiguousarray(np.asarray(inputs[key], np.float32))
    return shared


def _core_maps(inputs, shared):
    z = np.asarray(inputs["z"], np.float32)
    conn = np.asarray(inputs["connection"], np.float32)
    zT = [np.ascontiguousarray(z[b].T) for b in range(B)]
    cT = [np.ascontiguousarray(conn[b].T) for b in range(B)]
    in_maps = []
    for c in range(NCORES):
        b, tb = divmod(c, NTB)
        zr = np.roll(zT[b], -tb * TOK, axis=1)
        cr = np.roll(cT[b], -tb * TOK, axis=1)
        m = dict(shared)
        m["zb"] = _tile3(zr, _BF)
        m["cb"] = _tile3(cr, _BF)
        m["zf"] = _tile3(zr[:, 0:TOK], np.float32)
        m["cf"] = _tile3(cr[:, 0:TOK], np.float32)
        in_maps.append(m)
    return in_maps


def kernel(**inputs):
    z = np.asarray(inputs["z"], np.float32)
    dt_val = float(np.asarray(inputs["dt"]))
    temp_val = float(np.asarray(inputs["temp"]))

    key = (dt_val, temp_val)
    if key not in _CACHE:
        _CACHE[key] = build_program(dt_val, temp_val)
    nc = _CACHE[key]

    in_maps = _core_maps(inputs, _prep_shared(inputs))
    res = run_bass_kernel_spmd(nc, in_maps, list(range(NCORES)), **_RUN_KWARGS)
    global _LAST_RESULT
    _LAST_RESULT = res

    z2 = np.empty((B, L, D), np.float32)
    conn_new = np.empty((B, L, D), np.float32)
    for c in range(NCORES):
        b, tb = divmod(c, NTB)
        sl = slice(tb * TOK, (tb + 1) * TOK)
        z2[b, sl, :] = res.results[c]["z2o"].transpose(1, 0, 2).reshape(D, TOK).T
        conn_new[b, sl, :] = res.results[c]["cno"].transpose(1, 0, 2).reshape(D, TOK).T
    return z2, conn_new, z
